# revision 5
# baseline (speedup 1.0000x reference)
"""Trainium2 Bass kernel for nn_CGCoupler (segment_reduce) — v2.

The CG tables decompose into 147 block-ops out[bo] += c * x1[b1] * x2[b2]
over 64-wide (l,m) blocks; mirror symmetry collapses them to 78 terms over
16 output segments (<=6 terms/segment).

Layout: per core 512 rows -> [128 partitions, 16 blocks, 256 (=4 row
groups x 64 ch)] bf16, entry-major, so every DVE op streams 256-contiguous
inner runs.

Pipeline (all bf16):
 1. products on DVE tensor_tensor (2x mode): g11 raster, s11/a11 (trimmed),
    v12/v21^T, s12/a12 (affine-packed cells), g0x/gx0.
 2. S0 family has |c|=1: s01 = g0x+gx0 written DIRECTLY into tree slots;
    signs folded into a sign-split first tree level. g00 likewise.
 3. cg scaling: |value|-grouped tensor_scalar IMMEDIATE ops (4x mode) that
    read term cells and scatter into tree slots; negative signs absorbed by
    placing instances into the subtract-columns (j4,j5). Runs split across
    DVE and the otherwise-idle Scalar (ACT) engine.
 4. tree: slot = j*16+seg; L1: t1=sp[j1,j2]-sp[j4,j5] (one op) plus a
    3-way sign split for the (j0,j3) pair; L2/L3 contiguous adds.
 5. bf16 out DMA; host restores layout and casts to fp32.
"""
import numpy as np

N_CORES = 8
ROWS = 512
D = 1024
GN = 256          # 4 row-groups x 64 channels
NE = 57           # term entries
NSLOT = 96        # 6 j-columns x 16 segments

# ---- block-op tables (for the offline simulator / verification) ----
DIAG = [(0, 0), (1, 0), (1, 6), (1, 8), (2, 0), (2, 6), (3, 0), (3, 6), (3, 8)]
SYM = [
    (0, 1, 1, 1), (0, 2, 2, 1), (0, 3, 3, 1), (0, 4, 4, 1), (0, 5, 5, 1), (0, 6, 6, 1),
    (0, 7, 7, 1), (0, 8, 8, 1), (0, 9, 9, 1), (0, 10, 10, 1), (0, 11, 11, 1), (0, 12, 12, 1),
    (0, 13, 13, 1), (0, 14, 14, 1), (0, 15, 15, 1), (1, 2, 3, -1), (1, 2, 5, 1), (1, 3, 2, -1),
    (1, 3, 4, 1), (1, 4, 3, 1), (1, 4, 5, -1), (1, 4, 13, 1), (1, 4, 15, 1), (1, 5, 2, 1),
    (1, 5, 4, -1), (1, 5, 12, 1), (1, 5, 14, 1), (1, 6, 1, 1), (1, 6, 7, -1), (1, 6, 11, 1),
    (1, 7, 6, -1), (1, 7, 8, -1), (1, 7, 10, 1), (1, 8, 1, 1), (1, 8, 7, -1), (1, 8, 9, 1),
    (1, 8, 11, 1), (2, 3, 1, -1), (2, 3, 7, 1), (2, 4, 8, -1), (2, 4, 10, 1), (2, 5, 1, 1),
    (2, 5, 7, -1), (2, 5, 11, 1), (2, 6, 2, 1), (2, 6, 12, 1), (2, 7, 3, 1), (2, 7, 5, -1),
    (2, 7, 13, 1), (2, 8, 4, -1), (2, 8, 14, 1), (3, 4, 1, 1), (3, 4, 7, -1), (3, 4, 9, 1),
    (3, 4, 11, 1), (3, 5, 6, -1), (3, 5, 8, -1), (3, 5, 10, 1), (3, 6, 3, 1), (3, 6, 5, -1),
    (3, 6, 13, 1), (3, 7, 2, 1), (3, 7, 4, -1), (3, 7, 12, 1), (3, 7, 14, 1), (3, 8, 3, 1),
    (3, 8, 5, -1), (3, 8, 13, 1), (3, 8, 15, 1),
]
# S0 signs: c(0,b,b) = -1 for b in 1..3 and 9..15, +1 for b in 4..8.
S0_SIGN = {b: (1.0 if 4 <= b <= 8 else -1.0) for b in range(1, 16)}

# ---- term-tile entry layout ----
# t11: base 0, entry = 3*ma + mb          (cells of x1[1:4] x x2[1:4])
# s11: base 9 + idx of [(1,2),(1,3),(2,3)]
# a11: base 12 + same order
# s12: base 37, entry = 37 - 5*a' - 3*b'  (a'=a-1, b'=b-4)
# a12: base 38, entry = 38 + 7*a' + 1*b'
B_T11, B_S11, B_A11, B_S12, B_A12 = 0, 9, 12, 37, 38
S12_MAP = (-5, -3)
A12_MAP = (7, 1)
P11 = {(1, 2): 0, (1, 3): 1, (2, 3): 2}


def _entry(fam, a, b):
    if fam == 't11':
        return B_T11 + 3 * (a - 1) + (b - 1)
    if fam == 's11':
        return B_S11 + P11[(a, b)]
    if fam == 'a11':
        return B_A11 + P11[(a, b)]
    if fam == 's12':
        return B_S12 + S12_MAP[0] * (a - 1) + S12_MAP[1] * (b - 4)
    return B_A12 + A12_MAP[0] * (a - 1) + A12_MAP[1] * (b - 4)


# ---- expand runs: (imm, e0, s0, de, ds, k) ----
# slot = j*16+seg; columns j1..j3 enter the tree positively, j4..j5
# negatively, so slot-value = imm * T[entry] and sign(j) * imm = c.
RUNS3 = [
    (0.707107, 0, 72, 8, -48, 2),
    (0.707107, 9, 21, 1, -1, 2),
    (0.707107, 11, 23, 1, -4, 2),
    (0.707107, 13, 66, 1, -49, 2),
    (0.707107, 40, 71, 1, -49, 2),
    (0.707107, 15, 79, 10, -6, 2),
    (0.707107, 27, 89, 10, -58, 2),
    (0.707107, 53, 70, 1, -33, 2),
    (0.408248, 0, 86, 38, -33, 2),
    (0.408248, 39, 68, 2, 20, 2),
    (0.408248, 42, 39, 4, 16, 2),
    (0.408248, 48, 69, 4, 18, 2),
    (-0.408248, 8, 38, 45, 2, 2),
    (-0.408248, 55, 84, 1, 1, 2),
    (0.57735, 0, 16, 4, 16, 2),
    (0.57735, 24, 74, 4, 16, 2),
    (0.57735, 18, 78, 2, 16, 2),
    (-0.57735, 8, 64, 24, -38, 2),
    (0.57735, 34, 30, 0, 0, 1),
    (0.547723, 15, 67, 3, 15, 2),
    (0.547723, 23, 83, 2, -50, 2),
    (0.547723, 27, 65, 2, 16, 2),
    (-0.547723, 34, 18, 3, 17, 2),
    (0.182574, 15, 29, 10, 46, 2),
    (0.182574, 27, 27, 10, 18, 2),
    (0.816497, 4, 54, 41, 2, 2),
    (-0.816497, 49, 36, 0, 0, 1),
    (0.632456, 21, 77, 10, 14, 2),
    (-0.632456, 26, 34, 0, 0, 1),
    (0.447214, 18, 28, 16, 16, 2),
    (0.316228, 21, 51, 10, -2, 2),
    (-0.730297, 23, 61, 6, -18, 2),
    (0.774597, 26, 76, 0, 0, 1),
]


def _assigned_slots():
    s = set(range(16))            # j0 row: seg0 pad + S0 segs 1..15
    s.add(48)                     # g00 at j3 seg0
    for imm, e0, s0, de, ds, k in RUNS3:
        for t in range(k):
            sl = s0 + t * ds
            assert sl not in s, f"slot collision {sl}"
            s.add(sl)
    return s


PAD_SLOTS = sorted(set(range(NSLOT)) - _assigned_slots() | {0})


def _pad_runs():
    """Group pad slots into (start, stride, count) runs for memsets."""
    rem = sorted(set(PAD_SLOTS))
    runs = []
    while rem:
        if len(rem) == 1:
            runs.append((rem[0], 1, 1)); break
        best = None
        for i in range(len(rem)):
            for j in range(i + 1, len(rem)):
                d = rem[j] - rem[i]
                chain = [rem[i], rem[j]]
                while chain[-1] + d in rem:
                    chain.append(chain[-1] + d)
                if best is None or len(chain) > len(best[0]):
                    best = (chain, d)
        chain, d = best
        runs.append((chain[0], d, len(chain)))
        for x in chain:
            rem.remove(x)
    return runs


_CACHE = {}


def _build():
    from concourse import bacc, mybir
    from concourse.ap import AP
    import concourse.tile as tile

    bf16 = mybir.dt.bfloat16
    ALU = mybir.AluOpType

    nc = bacc.Bacc("TRN2", target_bir_lowering=False)
    x1_d = nc.dram_tensor("x1", [128, 16 * GN], bf16, kind="ExternalInput")
    x2_d = nc.dram_tensor("x2", [128, 16 * GN], bf16, kind="ExternalInput")
    out_d = nc.dram_tensor("out", [128, 16 * GN], bf16, kind="ExternalOutput")

    def sub(t, off, dims):
        base = t[:]
        return AP(base.tensor, base.offset + off, [list(base.ap[0])] + [[s, n] for s, n in dims])

    with tile.TileContext(nc) as tc:
        with tc.tile_pool(name="p", bufs=1) as p:
            x1b = p.tile([128, 16, GN], bf16)
            x2b = p.tile([128, 16, GN], bf16)
            term = p.tile([128, NE, GN], bf16)
            v12 = p.tile([128, 15, GN], bf16)
            v21 = p.tile([128, 15, GN], bf16)
            g0x = p.tile([128, 15, GN], bf16)
            gx0 = p.tile([128, 15, GN], bf16)
            sp = p.tile([128, NSLOT, GN], bf16)
            t1 = p.tile([128, 48, GN], bf16)
            u = p.tile([128, 16, GN], bf16)
            res = p.tile([128, 16, GN], bf16)

            # input waves: A = blocks 1..3, B = blocks 4..8, C = 0 and 9..15.
            # x2 rides the gpsimd (SWDGE) queue so the scalar queue's
            # ACT_TABLE_LOAD doesn't delay its descriptor generation.
            nc.sync.dma_start(x1b[:, 1:4], x1_d[:, GN:4 * GN])
            nc.gpsimd.dma_start(x2b[:, 1:4], x2_d[:, GN:4 * GN])
            nc.sync.dma_start(x1b[:, 4:9], x1_d[:, 4 * GN:9 * GN])
            nc.gpsimd.dma_start(x2b[:, 4:9], x2_d[:, 4 * GN:9 * GN])
            nc.sync.dma_start(x1b[:, 0:1], x1_d[:, 0:GN])
            nc.gpsimd.dma_start(x2b[:, 0:1], x2_d[:, 0:GN])
            nc.sync.dma_start(x1b[:, 9:16], x1_d[:, 9 * GN:16 * GN])
            nc.gpsimd.dma_start(x2b[:, 9:16], x2_d[:, 9 * GN:16 * GN])

            # pad slots -> 0 (gpsimd, no data deps, after its DMA issues)
            for (st, sd, cnt) in _pad_runs():
                nc.gpsimd.memset(sub(sp, st * GN, [(sd * GN, cnt), (1, GN)]), 0.0)

            # ---- products ----
            t11v = term[:, 0:9, :].rearrange("p (a b) n -> p a b n", a=3)
            nc.vector.tensor_tensor(
                t11v,
                x1b[:, 1:4, :].unsqueeze(2).to_broadcast([128, 3, 3, GN]),
                x2b[:, 1:4, :].unsqueeze(1).to_broadcast([128, 3, 3, GN]),
                op=ALU.mult)
            # s11/a11 for cells (1,2),(1,3),(2,3): t11 idx {1,2,5} + T {3,6,7}
            nc.vector.tensor_tensor(
                sub(term, B_S11 * GN, [(2 * GN, 2), (1, GN)]),
                sub(term, 1 * GN, [(4 * GN, 2), (1, GN)]),
                sub(term, 3 * GN, [(4 * GN, 2), (1, GN)]), op=ALU.add)
            nc.vector.tensor_tensor(
                term[:, B_S11 + 1:B_S11 + 2, :], term[:, 2:3, :], term[:, 6:7, :], op=ALU.add)
            nc.vector.tensor_tensor(
                sub(term, B_A11 * GN, [(2 * GN, 2), (1, GN)]),
                sub(term, 1 * GN, [(4 * GN, 2), (1, GN)]),
                sub(term, 3 * GN, [(4 * GN, 2), (1, GN)]), op=ALU.subtract)
            nc.vector.tensor_tensor(
                term[:, B_A11 + 1:B_A11 + 2, :], term[:, 2:3, :], term[:, 6:7, :], op=ALU.subtract)

            v12v = v12[:].rearrange("p (a b) n -> p a b n", a=3)
            v21v = v21[:].rearrange("p (a b) n -> p a b n", a=3)
            nc.vector.tensor_tensor(
                v12v,
                x1b[:, 1:4, :].unsqueeze(2).to_broadcast([128, 3, 5, GN]),
                x2b[:, 4:9, :].unsqueeze(1).to_broadcast([128, 3, 5, GN]),
                op=ALU.mult)
            # v21^T: cell (a',b') = x1[4+b'] * x2[1+a']
            nc.vector.tensor_tensor(
                v21v,
                x1b[:, 4:9, :].unsqueeze(1).to_broadcast([128, 3, 5, GN]),
                x2b[:, 1:4, :].unsqueeze(2).to_broadcast([128, 3, 5, GN]),
                op=ALU.mult)
            s12dst = sub(term, B_S12 * GN,
                         [(S12_MAP[0] * GN, 3), (S12_MAP[1] * GN, 5), (1, GN)])
            a12dst = sub(term, B_A12 * GN,
                         [(A12_MAP[0] * GN, 3), (A12_MAP[1] * GN, 5), (1, GN)])
            nc.vector.tensor_tensor(s12dst, v12v, v21v, op=ALU.add)
            nc.vector.tensor_tensor(a12dst, v12v, v21v, op=ALU.subtract)

            # g00 -> slot 48 (j3, seg0)
            nc.vector.tensor_tensor(sp[:, 48:49, :], x1b[:, 0:1, :], x2b[:, 0:1, :], op=ALU.mult)
            nc.vector.tensor_tensor(
                g0x[:], x1b[:, 0:1, :].to_broadcast([128, 15, GN]), x2b[:, 1:16, :], op=ALU.mult)
            nc.vector.tensor_tensor(
                gx0[:], x1b[:, 1:16, :], x2b[:, 0:1, :].to_broadcast([128, 15, GN]), op=ALU.mult)
            # s01 -> j0 slots 1..15 directly (|c| = 1; signs via L1 split)
            nc.vector.tensor_tensor(sp[:, 1:16, :], g0x[:], gx0[:], op=ALU.add)

            # ---- expand: |value|-grouped scaled scatter, spread over
            # DVE (fast) / ACT / GPSIMD (free engines) ----
            runs = []
            for (imm, e0, s0, de, ds, k) in RUNS3:
                maxe = e0 + (k - 1) * de if de > 0 else e0
                runs.append((maxe, imm, e0, s0, de, ds, k))
            runs.sort()
            n_act = n_gps = 0
            for idx, (maxe, imm, e0, s0, de, ds, k) in enumerate(runs):
                src = sub(term, e0 * GN, [(de * GN if k > 1 else GN, k), (1, GN)])
                dst = sub(sp, s0 * GN, [(ds * GN if k > 1 else GN, k), (1, GN)])
                if n_act < 19 and (maxe < 15 or idx % 3 != 0):
                    nc.scalar.mul(dst, src, float(imm))
                    n_act += 1
                elif n_gps < 6:
                    nc.gpsimd.tensor_scalar(dst, src, float(imm), None, op0=ALU.mult)
                    n_gps += 1
                else:
                    nc.vector.tensor_scalar(dst, src, float(imm), None, op0=ALU.mult)

            # ---- tree ----
            # (j0,j3) pair first (ready once s01/g00/j3-runs land), then the
            # big (j1,j2)-(j4,j5) subtract, then L2/L3 split for early DMA.
            nc.vector.tensor_tensor(t1[:, 0:4, :], sp[:, 48:52, :], sp[:, 0:4, :],
                                    op=ALU.subtract)
            nc.vector.tensor_tensor(t1[:, 4:9, :], sp[:, 52:57, :], sp[:, 4:9, :],
                                    op=ALU.add)
            nc.vector.tensor_tensor(t1[:, 9:16, :], sp[:, 57:64, :], sp[:, 9:16, :],
                                    op=ALU.subtract)
            nc.vector.tensor_tensor(t1[:, 16:48, :], sp[:, 16:48, :], sp[:, 64:96, :],
                                    op=ALU.subtract)
            nc.vector.tensor_tensor(u[:], t1[:, 0:16, :], t1[:, 16:32, :], op=ALU.add)
            for q in range(4):
                lo, hi = 4 * q, 4 * (q + 1)
                nc.vector.tensor_tensor(res[:, lo:hi, :], u[:, lo:hi, :],
                                        t1[:, 32 + lo:32 + hi, :], op=ALU.add)
                eng = nc.sync if q % 2 == 0 else nc.scalar
                eng.dma_start(out_d[:, lo * GN:hi * GN],
                              res[:, lo:hi, :].rearrange("p s n -> p (s n)"))

    nc.compile()
    return nc


def _get_nc():
    if "nc" not in _CACHE:
        _CACHE["nc"] = _build()
    return _CACHE["nc"]


def _in_maps(np_inputs):
    import ml_dtypes
    x1 = np.asarray(np_inputs["x1"], dtype=np.float32).astype(ml_dtypes.bfloat16)
    x2 = np.asarray(np_inputs["x2"], dtype=np.float32).astype(ml_dtypes.bfloat16)
    n = x1.shape[0]
    rows = n // N_CORES
    in_maps = []
    for k in range(N_CORES):
        sl = slice(k * rows, (k + 1) * rows)
        # [512, 1024] -> [g, p, blk, ch] -> [p, blk, g, ch] -> [128, 4096]
        a1 = x1[sl].reshape(4, 128, 16, 64).transpose(1, 2, 0, 3).reshape(128, 4096)
        a2 = x2[sl].reshape(4, 128, 16, 64).transpose(1, 2, 0, 3).reshape(128, 4096)
        in_maps.append({"x1": np.ascontiguousarray(a1),
                        "x2": np.ascontiguousarray(a2)})
    return in_maps


def kernel(x1, x2, cg_tilde, repids_in1, repids_in2, repids_out, out_dim):
    from concourse.bass_utils import run_bass_kernel_spmd

    nc = _get_nc()
    in_maps = _in_maps({"x1": x1, "x2": x2})
    res = run_bass_kernel_spmd(nc, in_maps, core_ids=list(range(N_CORES)))
    outs = []
    for k in range(N_CORES):
        r = np.asarray(res.results[k]["out"])
        # [128, 4096] -> [p, seg, g, ch] -> [g, p, seg, ch] -> [512, 1024]
        outs.append(r.reshape(128, 16, 4, 64).transpose(2, 0, 1, 3)
                    .reshape(512, 1024).astype(np.float32))
    return np.concatenate(outs, axis=0)


# ---------- offline simulator (dev aid; mirrors the on-chip math) ----------
def _simulate(x1, x2):
    """numpy fp32 simulation of the kernel's index plan on full inputs."""
    n = x1.shape[0]
    xb1 = x1.reshape(n, 16, 64)
    xb2 = x2.reshape(n, 16, 64)
    term = np.zeros((n, NE, 64), np.float32)
    for a in range(1, 4):
        for b in range(1, 4):
            term[:, 3 * (a - 1) + (b - 1)] = xb1[:, a] * xb2[:, b]
    for (a, b), i in P11.items():
        ab = xb1[:, a] * xb2[:, b] + xb1[:, b] * xb2[:, a]
        term[:, B_S11 + i] = ab
        term[:, B_A11 + i] = xb1[:, a] * xb2[:, b] - xb1[:, b] * xb2[:, a]
    for a in range(1, 4):
        for b in range(4, 9):
            s = xb1[:, a] * xb2[:, b] + xb1[:, b] * xb2[:, a]
            d = xb1[:, a] * xb2[:, b] - xb1[:, b] * xb2[:, a]
            term[:, _entry('s12', a, b)] = s
            term[:, _entry('a12', a, b)] = d
    sp = np.zeros((n, NSLOT, 64), np.float32)
    sp[:, 48] = xb1[:, 0] * xb2[:, 0]
    for b in range(1, 16):
        sp[:, b] = xb1[:, 0] * xb2[:, b] + xb1[:, b] * xb2[:, 0]
    for (imm, e0, s0, de, ds, k) in RUNS3:
        for t in range(k):
            sp[:, s0 + t * ds] = imm * term[:, e0 + t * de]
    t1 = np.zeros((n, 48, 64), np.float32)
    t1[:, 16:48] = sp[:, 16:48] - sp[:, 64:96]
    t1[:, 0:4] = sp[:, 48:52] - sp[:, 0:4]
    t1[:, 4:9] = sp[:, 52:57] + sp[:, 4:9]
    t1[:, 9:16] = sp[:, 57:64] - sp[:, 9:16]
    uu = t1[:, 0:16] + t1[:, 16:32]
    rr = uu + t1[:, 32:48]
    return rr.reshape(n, 1024)


# revision 6
# speedup vs baseline: 1.8238x; 1.8238x over previous
"""Trainium2 Bass kernel for nn_CGCoupler (segment_reduce) — v2.

The CG tables decompose into 147 block-ops out[bo] += c * x1[b1] * x2[b2]
over 64-wide (l,m) blocks; mirror symmetry collapses them to 78 terms over
16 output segments (<=6 terms/segment).

Layout: per core 512 rows -> [128 partitions, 16 blocks, 256 (=4 row
groups x 64 ch)] bf16, entry-major, so every DVE op streams 256-contiguous
inner runs.

Pipeline (all bf16):
 1. products on DVE tensor_tensor (2x mode): g11 raster, s11/a11 (trimmed),
    v12/v21^T, s12/a12 (affine-packed cells), g0x/gx0.
 2. S0 family has |c|=1: s01 = g0x+gx0 written DIRECTLY into tree slots;
    signs folded into a sign-split first tree level. g00 likewise.
 3. cg scaling: |value|-grouped tensor_scalar IMMEDIATE ops (4x mode) that
    read term cells and scatter into tree slots; negative signs absorbed by
    placing instances into the subtract-columns (j4,j5). Runs split across
    DVE and the otherwise-idle Scalar (ACT) engine.
 4. tree: slot = j*16+seg; L1: t1=sp[j1,j2]-sp[j4,j5] (one op) plus a
    3-way sign split for the (j0,j3) pair; L2/L3 contiguous adds.
 5. bf16 out DMA; host restores layout and casts to fp32.
"""
import numpy as np

N_CORES = 8
ROWS = 512
D = 1024
GN = 256          # 4 row-groups x 64 channels
NE = 57           # term entries
NSLOT = 96        # 6 j-columns x 16 segments

# ---- block-op tables (for the offline simulator / verification) ----
DIAG = [(0, 0), (1, 0), (1, 6), (1, 8), (2, 0), (2, 6), (3, 0), (3, 6), (3, 8)]
SYM = [
    (0, 1, 1, 1), (0, 2, 2, 1), (0, 3, 3, 1), (0, 4, 4, 1), (0, 5, 5, 1), (0, 6, 6, 1),
    (0, 7, 7, 1), (0, 8, 8, 1), (0, 9, 9, 1), (0, 10, 10, 1), (0, 11, 11, 1), (0, 12, 12, 1),
    (0, 13, 13, 1), (0, 14, 14, 1), (0, 15, 15, 1), (1, 2, 3, -1), (1, 2, 5, 1), (1, 3, 2, -1),
    (1, 3, 4, 1), (1, 4, 3, 1), (1, 4, 5, -1), (1, 4, 13, 1), (1, 4, 15, 1), (1, 5, 2, 1),
    (1, 5, 4, -1), (1, 5, 12, 1), (1, 5, 14, 1), (1, 6, 1, 1), (1, 6, 7, -1), (1, 6, 11, 1),
    (1, 7, 6, -1), (1, 7, 8, -1), (1, 7, 10, 1), (1, 8, 1, 1), (1, 8, 7, -1), (1, 8, 9, 1),
    (1, 8, 11, 1), (2, 3, 1, -1), (2, 3, 7, 1), (2, 4, 8, -1), (2, 4, 10, 1), (2, 5, 1, 1),
    (2, 5, 7, -1), (2, 5, 11, 1), (2, 6, 2, 1), (2, 6, 12, 1), (2, 7, 3, 1), (2, 7, 5, -1),
    (2, 7, 13, 1), (2, 8, 4, -1), (2, 8, 14, 1), (3, 4, 1, 1), (3, 4, 7, -1), (3, 4, 9, 1),
    (3, 4, 11, 1), (3, 5, 6, -1), (3, 5, 8, -1), (3, 5, 10, 1), (3, 6, 3, 1), (3, 6, 5, -1),
    (3, 6, 13, 1), (3, 7, 2, 1), (3, 7, 4, -1), (3, 7, 12, 1), (3, 7, 14, 1), (3, 8, 3, 1),
    (3, 8, 5, -1), (3, 8, 13, 1), (3, 8, 15, 1),
]
# S0 signs: c(0,b,b) = -1 for b in 1..3 and 9..15, +1 for b in 4..8.
S0_SIGN = {b: (1.0 if 4 <= b <= 8 else -1.0) for b in range(1, 16)}

# ---- term-tile entry layout ----
# t11: base 0, entry = 3*ma + mb          (cells of x1[1:4] x x2[1:4])
# s11: base 9 + idx of [(1,2),(1,3),(2,3)]
# a11: base 12 + same order
# s12: base 37, entry = 37 - 5*a' - 3*b'  (a'=a-1, b'=b-4)
# a12: base 38, entry = 38 + 7*a' + 1*b'
B_T11, B_S11, B_A11, B_S12, B_A12 = 0, 9, 12, 37, 38
S12_MAP = (-5, -3)
A12_MAP = (7, 1)
P11 = {(1, 2): 0, (1, 3): 1, (2, 3): 2}


def _entry(fam, a, b):
    if fam == 't11':
        return B_T11 + 3 * (a - 1) + (b - 1)
    if fam == 's11':
        return B_S11 + P11[(a, b)]
    if fam == 'a11':
        return B_A11 + P11[(a, b)]
    if fam == 's12':
        return B_S12 + S12_MAP[0] * (a - 1) + S12_MAP[1] * (b - 4)
    return B_A12 + A12_MAP[0] * (a - 1) + A12_MAP[1] * (b - 4)


# ---- expand runs: (imm, e0, s0, de, ds, k) ----
# slot = j*16+seg; columns j1..j3 enter the tree positively, j4..j5
# negatively, so slot-value = imm * T[entry] and sign(j) * imm = c.
RUNS3 = [
    (0.707107, 0, 72, 8, -48, 2),
    (0.707107, 9, 21, 1, -1, 2),
    (0.707107, 11, 23, 1, -4, 2),
    (0.707107, 13, 66, 1, -49, 2),
    (0.707107, 40, 71, 1, -49, 2),
    (0.707107, 15, 79, 10, -6, 2),
    (0.707107, 27, 89, 10, -58, 2),
    (0.707107, 53, 70, 1, -33, 2),
    (0.408248, 0, 86, 38, -33, 2),
    (0.408248, 39, 68, 2, 20, 2),
    (0.408248, 42, 39, 4, 16, 2),
    (0.408248, 48, 69, 4, 18, 2),
    (-0.408248, 8, 38, 45, 2, 2),
    (-0.408248, 55, 84, 1, 1, 2),
    (0.57735, 0, 16, 4, 16, 2),
    (0.57735, 24, 74, 4, 16, 2),
    (0.57735, 18, 78, 2, 16, 2),
    (-0.57735, 8, 64, 24, -38, 2),
    (0.57735, 34, 30, 0, 0, 1),
    (0.547723, 15, 67, 3, 15, 2),
    (0.547723, 23, 83, 2, -50, 2),
    (0.547723, 27, 65, 2, 16, 2),
    (-0.547723, 34, 18, 3, 17, 2),
    (0.182574, 15, 29, 10, 46, 2),
    (0.182574, 27, 27, 10, 18, 2),
    (0.816497, 4, 54, 41, 2, 2),
    (-0.816497, 49, 36, 0, 0, 1),
    (0.632456, 21, 77, 10, 14, 2),
    (-0.632456, 26, 34, 0, 0, 1),
    (0.447214, 18, 28, 16, 16, 2),
    (0.316228, 21, 51, 10, -2, 2),
    (-0.730297, 23, 61, 6, -18, 2),
    (0.774597, 26, 76, 0, 0, 1),
]


def _assigned_slots():
    s = set(range(16))            # j0 row: seg0 pad + S0 segs 1..15
    s.add(48)                     # g00 at j3 seg0
    for imm, e0, s0, de, ds, k in RUNS3:
        for t in range(k):
            sl = s0 + t * ds
            assert sl not in s, f"slot collision {sl}"
            s.add(sl)
    return s


PAD_SLOTS = sorted(set(range(NSLOT)) - _assigned_slots() | {0})


def _pad_runs():
    """Group pad slots into (start, stride, count) runs for memsets."""
    rem = sorted(set(PAD_SLOTS))
    runs = []
    while rem:
        if len(rem) == 1:
            runs.append((rem[0], 1, 1)); break
        best = None
        for i in range(len(rem)):
            for j in range(i + 1, len(rem)):
                d = rem[j] - rem[i]
                chain = [rem[i], rem[j]]
                while chain[-1] + d in rem:
                    chain.append(chain[-1] + d)
                if best is None or len(chain) > len(best[0]):
                    best = (chain, d)
        chain, d = best
        runs.append((chain[0], d, len(chain)))
        for x in chain:
            rem.remove(x)
    return runs


_CACHE = {}


def _build():
    from concourse import bacc, mybir
    from concourse.ap import AP
    import concourse.tile as tile

    bf16 = mybir.dt.bfloat16
    ALU = mybir.AluOpType

    nc = bacc.Bacc("TRN2", target_bir_lowering=False)
    x1_d = nc.dram_tensor("x1", [128, 16 * GN], bf16, kind="ExternalInput")
    x2_d = nc.dram_tensor("x2", [128, 16 * GN], bf16, kind="ExternalInput")
    out_d = nc.dram_tensor("out", [128, 16 * GN], bf16, kind="ExternalOutput")

    def sub(t, off, dims):
        base = t[:]
        return AP(base.tensor, base.offset + off, [list(base.ap[0])] + [[s, n] for s, n in dims])

    with tile.TileContext(nc) as tc:
        with tc.tile_pool(name="p", bufs=1) as p:
            x1b = p.tile([128, 16, GN], bf16)
            x2b = p.tile([128, 16, GN], bf16)
            term = p.tile([128, NE, GN], bf16)
            v12 = p.tile([128, 15, GN], bf16)
            v21 = p.tile([128, 15, GN], bf16)
            g0x = p.tile([128, 15, GN], bf16)
            gx0 = p.tile([128, 15, GN], bf16)
            sp = p.tile([128, NSLOT, GN], bf16)
            t1 = p.tile([128, 48, GN], bf16)
            u = p.tile([128, 16, GN], bf16)
            res = p.tile([128, 16, GN], bf16)

            # input waves: A = blocks 1..3, B = blocks 4..8, C = 0 and 9..15.
            # x2 rides the gpsimd (SWDGE) queue so the scalar queue's
            # ACT_TABLE_LOAD doesn't delay its descriptor generation.
            nc.sync.dma_start(x1b[:, 1:4], x1_d[:, GN:4 * GN])
            nc.gpsimd.dma_start(x2b[:, 1:4], x2_d[:, GN:4 * GN])
            nc.sync.dma_start(x1b[:, 4:9], x1_d[:, 4 * GN:9 * GN])
            nc.gpsimd.dma_start(x2b[:, 4:9], x2_d[:, 4 * GN:9 * GN])
            nc.sync.dma_start(x1b[:, 0:1], x1_d[:, 0:GN])
            nc.gpsimd.dma_start(x2b[:, 0:1], x2_d[:, 0:GN])
            nc.sync.dma_start(x1b[:, 9:16], x1_d[:, 9 * GN:16 * GN])
            nc.gpsimd.dma_start(x2b[:, 9:16], x2_d[:, 9 * GN:16 * GN])

            # pad slots -> 0 (gpsimd, no data deps, after its DMA issues)
            for (st, sd, cnt) in _pad_runs():
                nc.gpsimd.memset(sub(sp, st * GN, [(sd * GN, cnt), (1, GN)]), 0.0)

            # ---- products ----
            t11v = term[:, 0:9, :].rearrange("p (a b) n -> p a b n", a=3)
            nc.vector.tensor_tensor(
                t11v,
                x1b[:, 1:4, :].unsqueeze(2).to_broadcast([128, 3, 3, GN]),
                x2b[:, 1:4, :].unsqueeze(1).to_broadcast([128, 3, 3, GN]),
                op=ALU.mult)
            # s11/a11 for cells (1,2),(1,3),(2,3): t11 idx {1,2,5} + T {3,6,7}
            nc.vector.tensor_tensor(
                sub(term, B_S11 * GN, [(2 * GN, 2), (1, GN)]),
                sub(term, 1 * GN, [(4 * GN, 2), (1, GN)]),
                sub(term, 3 * GN, [(4 * GN, 2), (1, GN)]), op=ALU.add)
            nc.vector.tensor_tensor(
                term[:, B_S11 + 1:B_S11 + 2, :], term[:, 2:3, :], term[:, 6:7, :], op=ALU.add)
            nc.vector.tensor_tensor(
                sub(term, B_A11 * GN, [(2 * GN, 2), (1, GN)]),
                sub(term, 1 * GN, [(4 * GN, 2), (1, GN)]),
                sub(term, 3 * GN, [(4 * GN, 2), (1, GN)]), op=ALU.subtract)
            nc.vector.tensor_tensor(
                term[:, B_A11 + 1:B_A11 + 2, :], term[:, 2:3, :], term[:, 6:7, :], op=ALU.subtract)

            v12v = v12[:].rearrange("p (a b) n -> p a b n", a=3)
            v21v = v21[:].rearrange("p (a b) n -> p a b n", a=3)
            nc.vector.tensor_tensor(
                v12v,
                x1b[:, 1:4, :].unsqueeze(2).to_broadcast([128, 3, 5, GN]),
                x2b[:, 4:9, :].unsqueeze(1).to_broadcast([128, 3, 5, GN]),
                op=ALU.mult)
            # v21^T: cell (a',b') = x1[4+b'] * x2[1+a']
            nc.vector.tensor_tensor(
                v21v,
                x1b[:, 4:9, :].unsqueeze(1).to_broadcast([128, 3, 5, GN]),
                x2b[:, 1:4, :].unsqueeze(2).to_broadcast([128, 3, 5, GN]),
                op=ALU.mult)
            s12dst = sub(term, B_S12 * GN,
                         [(S12_MAP[0] * GN, 3), (S12_MAP[1] * GN, 5), (1, GN)])
            a12dst = sub(term, B_A12 * GN,
                         [(A12_MAP[0] * GN, 3), (A12_MAP[1] * GN, 5), (1, GN)])
            nc.vector.tensor_tensor(s12dst, v12v, v21v, op=ALU.add)
            nc.vector.tensor_tensor(a12dst, v12v, v21v, op=ALU.subtract)

            # g00 -> slot 48 (j3, seg0)
            nc.vector.tensor_tensor(sp[:, 48:49, :], x1b[:, 0:1, :], x2b[:, 0:1, :], op=ALU.mult)
            nc.vector.tensor_tensor(
                g0x[:], x1b[:, 0:1, :].to_broadcast([128, 15, GN]), x2b[:, 1:16, :], op=ALU.mult)
            nc.vector.tensor_tensor(
                gx0[:], x1b[:, 1:16, :], x2b[:, 0:1, :].to_broadcast([128, 15, GN]), op=ALU.mult)
            # s01 -> j0 slots 1..15 directly (|c| = 1; signs via L1 split)
            nc.vector.tensor_tensor(sp[:, 1:16, :], g0x[:], gx0[:], op=ALU.add)

            # ---- expand: |value|-grouped scaled scatter, spread over
            # DVE (fast) / ACT / GPSIMD (free engines) ----
            runs = []
            for (imm, e0, s0, de, ds, k) in RUNS3:
                maxe = e0 + (k - 1) * de if de > 0 else e0
                runs.append((maxe, imm, e0, s0, de, ds, k))
            runs.sort()
            n_act = n_gps = 0
            for idx, (maxe, imm, e0, s0, de, ds, k) in enumerate(runs):
                src = sub(term, e0 * GN, [(de * GN if k > 1 else GN, k), (1, GN)])
                dst = sub(sp, s0 * GN, [(ds * GN if k > 1 else GN, k), (1, GN)])
                if n_act < 19 and (maxe < 15 or idx % 3 != 0):
                    nc.scalar.mul(dst, src, float(imm))
                    n_act += 1
                else:
                    nc.vector.tensor_scalar(dst, src, float(imm), None, op0=ALU.mult)

            # ---- tree ----
            # (j0,j3) pair first (ready once s01/g00/j3-runs land), then the
            # big (j1,j2)-(j4,j5) subtract, then L2/L3 split for early DMA.
            nc.vector.tensor_tensor(t1[:, 0:4, :], sp[:, 48:52, :], sp[:, 0:4, :],
                                    op=ALU.subtract)
            nc.vector.tensor_tensor(t1[:, 4:9, :], sp[:, 52:57, :], sp[:, 4:9, :],
                                    op=ALU.add)
            nc.vector.tensor_tensor(t1[:, 9:16, :], sp[:, 57:64, :], sp[:, 9:16, :],
                                    op=ALU.subtract)
            nc.vector.tensor_tensor(t1[:, 16:48, :], sp[:, 16:48, :], sp[:, 64:96, :],
                                    op=ALU.subtract)
            nc.vector.tensor_tensor(u[:], t1[:, 0:16, :], t1[:, 16:32, :], op=ALU.add)
            for q in range(4):
                lo, hi = 4 * q, 4 * (q + 1)
                nc.vector.tensor_tensor(res[:, lo:hi, :], u[:, lo:hi, :],
                                        t1[:, 32 + lo:32 + hi, :], op=ALU.add)
                eng = nc.sync if q % 2 == 0 else nc.scalar
                eng.dma_start(out_d[:, lo * GN:hi * GN],
                              res[:, lo:hi, :].rearrange("p s n -> p (s n)"))

    nc.compile()
    return nc


def _get_nc():
    if "nc" not in _CACHE:
        _CACHE["nc"] = _build()
    return _CACHE["nc"]


def _in_maps(np_inputs):
    import ml_dtypes
    x1 = np.asarray(np_inputs["x1"], dtype=np.float32).astype(ml_dtypes.bfloat16)
    x2 = np.asarray(np_inputs["x2"], dtype=np.float32).astype(ml_dtypes.bfloat16)
    n = x1.shape[0]
    rows = n // N_CORES
    in_maps = []
    for k in range(N_CORES):
        sl = slice(k * rows, (k + 1) * rows)
        # [512, 1024] -> [g, p, blk, ch] -> [p, blk, g, ch] -> [128, 4096]
        a1 = x1[sl].reshape(4, 128, 16, 64).transpose(1, 2, 0, 3).reshape(128, 4096)
        a2 = x2[sl].reshape(4, 128, 16, 64).transpose(1, 2, 0, 3).reshape(128, 4096)
        in_maps.append({"x1": np.ascontiguousarray(a1),
                        "x2": np.ascontiguousarray(a2)})
    return in_maps


def kernel(x1, x2, cg_tilde, repids_in1, repids_in2, repids_out, out_dim):
    from concourse.bass_utils import run_bass_kernel_spmd

    nc = _get_nc()
    in_maps = _in_maps({"x1": x1, "x2": x2})
    res = run_bass_kernel_spmd(nc, in_maps, core_ids=list(range(N_CORES)))
    outs = []
    for k in range(N_CORES):
        r = np.asarray(res.results[k]["out"])
        # [128, 4096] -> [p, seg, g, ch] -> [g, p, seg, ch] -> [512, 1024]
        outs.append(r.reshape(128, 16, 4, 64).transpose(2, 0, 1, 3)
                    .reshape(512, 1024).astype(np.float32))
    return np.concatenate(outs, axis=0)


# ---------- offline simulator (dev aid; mirrors the on-chip math) ----------
def _simulate(x1, x2):
    """numpy fp32 simulation of the kernel's index plan on full inputs."""
    n = x1.shape[0]
    xb1 = x1.reshape(n, 16, 64)
    xb2 = x2.reshape(n, 16, 64)
    term = np.zeros((n, NE, 64), np.float32)
    for a in range(1, 4):
        for b in range(1, 4):
            term[:, 3 * (a - 1) + (b - 1)] = xb1[:, a] * xb2[:, b]
    for (a, b), i in P11.items():
        ab = xb1[:, a] * xb2[:, b] + xb1[:, b] * xb2[:, a]
        term[:, B_S11 + i] = ab
        term[:, B_A11 + i] = xb1[:, a] * xb2[:, b] - xb1[:, b] * xb2[:, a]
    for a in range(1, 4):
        for b in range(4, 9):
            s = xb1[:, a] * xb2[:, b] + xb1[:, b] * xb2[:, a]
            d = xb1[:, a] * xb2[:, b] - xb1[:, b] * xb2[:, a]
            term[:, _entry('s12', a, b)] = s
            term[:, _entry('a12', a, b)] = d
    sp = np.zeros((n, NSLOT, 64), np.float32)
    sp[:, 48] = xb1[:, 0] * xb2[:, 0]
    for b in range(1, 16):
        sp[:, b] = xb1[:, 0] * xb2[:, b] + xb1[:, b] * xb2[:, 0]
    for (imm, e0, s0, de, ds, k) in RUNS3:
        for t in range(k):
            sp[:, s0 + t * ds] = imm * term[:, e0 + t * de]
    t1 = np.zeros((n, 48, 64), np.float32)
    t1[:, 16:48] = sp[:, 16:48] - sp[:, 64:96]
    t1[:, 0:4] = sp[:, 48:52] - sp[:, 0:4]
    t1[:, 4:9] = sp[:, 52:57] + sp[:, 4:9]
    t1[:, 9:16] = sp[:, 57:64] - sp[:, 9:16]
    uu = t1[:, 0:16] + t1[:, 16:32]
    rr = uu + t1[:, 32:48]
    return rr.reshape(n, 1024)
